# revision 40
# baseline (speedup 1.0000x reference)
"""Trainium2 Bass kernel for nn_CumulativeFFT.

out[b,t,d,k,c] = (1/sqrt(2048)) * cumsum_t( x[b,t,d] * tw[t,k,c] )

Sharding: 8 cores = batch(4) x time-half(2). Each core computes its
(1024, 256, 32, 2) output shard; the cross-half running offset is computed
on-device from an x_prev input (zeros for first-half cores, so the program
stays SPMD-uniform).

Per-core algorithm (T on 126-row blocks, m-major contribution layout):
  - Contributions C[s, m*256+d] = x[s,d] * tw[s,m] built by per-m
    tensor_scalar ops (per-partition scalar = twiddle column): m 0..39
    on DVE (4x mode, ~127ns/op), m 40..63 on GPSIMD (~450ns/op). C is
    split into two half-width tiles (m<32 / m>=32) rotating 2-deep.
  - Causal cumsum via two PE matmuls per 1024-col psum tile with a
    constant lhsT: rows s<126 = upper-tri ones, row 126 = all-ones
    "carry row" holding the running carry in bf16 (the fp32 scan is
    rounded once per block; rel err stays ~4e-3, well inside the 2e-2
    gate). Matmul APs use 127 rows so the unwritten row 127 is never
    read. The 16-row remainder block keeps its carry at row 16 with a
    17-row lhsT slice.
  - Block carries from tiny blocksum matmuls tw_blk^T @ x_blk that
    ping-pong two psum tiles and are copied to SBUF fp32 on ACT right
    away (so their psum-ring slots free early); the serial scan adds
    run on DVE and the bf16 carry-row copies on Pool, both
    deprioritized below the build stream. GPSIMD cannot access PSUM on
    this hardware, which forces this engine split.
  - PSUM -> bf16 convert (x bf16(1/sqrt(2048))): ACT takes 11 of the 16
    psum tiles per block, DVE takes tiles {0,2,4,6,9} (placed early/
    spread so DVE drains its psum-ring slots near the ring's pace after
    finishing builds). The m-major -> d-major reorder is free via the
    strided convert APs. Per-block engine work (DVE ~11.0us, ACT
    ~11.4us, Pool ~11.3us) sits just under the 11.6us/block store time.
  - A warmup burst of dummy matmuls ramps the PE p-state while the
    input DMAs run; the cross-half offset chain (ACT) then feeds the
    block-0 carry row directly from psum.
  - Carry-row DMAs are emitted TWO blocks ahead of their stores in
    SP-queue order: an SP DMA holds the queue for its whole transfer,
    so a carry DMA emitted late would wait ~11.5us behind a block of
    stores, gating every matmul of its block (this was the dominant
    stall in earlier versions).

The kernel is DMA-bound in steady state: stores stream back-to-back at
the 360GB/s per-core roofline (32MB of output = ~93us); all remaining
slack is the ~33us pipeline fill/drain at the ends.
"""

import math
import sys

import numpy as np

sys.path.insert(0, "/opt/trn_rl_repo")

import ml_dtypes

BF16 = ml_dtypes.bfloat16

B, T, D, K = 4, 2048, 256, 32
M2 = 2 * K            # 64 (k,c) pairs
MH = M2 // 2          # 32 m's per C half-tile
NCORES = 8
TH = T // 2           # 1024 time steps per core
TB = 126              # time-block rows (partitions 0..125; 126/127 = carry)
NFULL = TH // TB      # 8
REM = TH - NFULL * TB # 16
NBLK = NFULL + 1      # 9
NPREV = TH // 128     # 8 (128-row blocks of the other half, for the offset)
WID = M2 * D          # 16384 = m-major row width (col = m*D + d)
WH = MH * D           # 8192 = half-tile width
PS_FREE = 1024        # psum tile free width: 4 m's = 2 banks (ring depth 4)
NORM = float(np.float32(BF16(1.0 / math.sqrt(T))))
NWARM = 24           # PE warmup matmuls (free-128, ramp p-state during loads)

# Convert-engine split per block (16 psum tiles): ACT takes 11, DVE takes 5
# (GPSIMD cannot read PSUM on real hardware, so Pool does C-builds instead).
# The DVE converts are woven into the next block's build stream at spread
# positions so they drain their psum-ring slots roughly on the ring's pace.
D_TILES = (0, 2, 4, 6, 9)
CONV_ENG = "".join("D" if n in D_TILES else "A" for n in range(16))
BD = 40               # builds on DVE (m 0..39); the rest on Pool
# build index -> which pending DVE convert to emit after it
WEAVE = {12: 0, 24: 1, 36: 2, 48: 3, 60: 4}

DISABLE_WEAVE = True

_prog = None


def _twiddles_np():
    n = np.arange(T, dtype=np.float32)
    k = np.arange(K, dtype=np.float32)
    ang = np.float32(-2.0 * math.pi / T) * np.outer(n, k)   # (T, K) f32
    tw = np.stack([np.cos(ang), np.sin(ang)], axis=-1)       # (T, K, 2)
    return tw.reshape(T, M2).astype(BF16)                    # m = k*2 + c


def _build_program():
    import concourse.bass as bass
    import concourse.tile as tile
    from concourse import bacc, mybir

    ts = bass.ts
    bf = mybir.dt.bfloat16
    f32 = mybir.dt.float32

    nc = bacc.Bacc(
        "TRN2", target_bir_lowering=False, debug=False, num_devices=NCORES
    )
    xo_h = nc.dram_tensor("x_own", [TH, D], bf, kind="ExternalInput")
    xp_h = nc.dram_tensor("x_prev", [TH, D], bf, kind="ExternalInput")
    two_h = nc.dram_tensor("tw_own", [128, NBLK * M2], bf, kind="ExternalInput")
    two32_h = nc.dram_tensor(
        "tw_own32", [128, NBLK * M2], f32, kind="ExternalInput"
    )
    twp_h = nc.dram_tensor("tw_prev", [128, NPREV * M2], bf, kind="ExternalInput")
    ut_h = nc.dram_tensor("ut", [128, TB + REM], bf, kind="ExternalInput")
    out_h = nc.dram_tensor("out", [TH, WID], bf, kind="ExternalOutput")

    with tile.TileContext(nc) as tc:
        with (
            tc.tile_pool(name="const", bufs=1) as cpool,
            tc.tile_pool(name="carry", bufs=1) as carpool,
            tc.tile_pool(name="cbuf", bufs=2) as cbpool,
            tc.tile_pool(name="obuf", bufs=3) as obpool,
            tc.tile_pool(name="ps", bufs=4, space="PSUM") as pspool,
        ):
            # PE warmup: ramp the p-state while input DMAs stream in.
            warm = cpool.tile([128, 512], bf, tag="warm")
            nc.vector.memset(warm[:, :], 0)
            # Dummy ACT op: hoists the implicit 1.3us activation-table load
            # to t~0 so it isn't paid on the offset->carry(0) chain later.
            nc.scalar.copy(warm[0:1, 256:260], warm[0:1, 0:4])
            ps_w = pspool.tile([128, PS_FREE], f32, tag="ps")
            for _ in range(NWARM):
                nc.tensor.matmul(
                    ps_w[:, 0:128], warm[:, 0:128], warm[:, 0:128],
                    start=True, stop=True,
                )

            # Loads: x_prev/tw_prev first (they gate the offset->carry(0)
            # chain, ~2.5us of latency after landing); x_own block 0 and
            # tw32 next so DVE builds start ASAP; the rest stream behind.
            xp_wide = cpool.tile([128, NPREV * D], bf, tag="xpw")
            nc.sync.dma_start(
                xp_wide[:, :],
                xp_h[:, :].rearrange("(i p) d -> p i d", p=128),
            )
            twp_t = cpool.tile([128, NPREV * M2], bf, tag="twp")
            nc.sync.dma_start(twp_t[:], twp_h[:])
            two32_t = cpool.tile([128, NBLK * M2], f32, tag="two32")
            nc.sync.dma_start(two32_t[:], two32_h[:])
            xo_wide = cpool.tile([128, NFULL * D], bf, tag="xow")
            nc.sync.dma_start(
                xo_wide[0:TB, 0:D],
                xo_h[0:TB, :],
            )
            two_t = cpool.tile([128, NBLK * M2], bf, tag="two")
            nc.sync.dma_start(two_t[:], two_h[:])
            # x blocks 1-3 ahead of ut: they feed the blocksum chain, whose
            # ACT-copy-paced serial links otherwise start ~1us later.
            nc.sync.dma_start(
                xo_wide[0:TB, D : 4 * D].rearrange("p (j d) -> p j d", d=D),
                xo_h[TB : 4 * TB, :].rearrange("(j p) d -> p j d", p=TB),
            )
            ut_t = cpool.tile([128, TB + REM], bf, tag="ut")
            nc.sync.dma_start(ut_t[:], ut_h[:])
            nc.sync.dma_start(
                xo_wide[0:TB, 4 * D :].rearrange("p (j d) -> p j d", d=D),
                xo_h[4 * TB : NFULL * TB, :].rearrange("(j p) d -> p j d", p=TB),
            )
            xo_rem = cpool.tile([128, D], bf, tag="xor")
            nc.sync.dma_start(xo_rem[0:REM, :], xo_h[NFULL * TB : TH, :])
            xo_tiles = [
                xo_wide[:, ts(j, D)] for j in range(NFULL)
            ] + [xo_rem[:, :]]
            xp_tiles = [xp_wide[:, ts(i, D)] for i in range(NPREV)]

            # ---- carry state ----
            # Single bf16 carry row per block (C row 126): the fp32 scan is
            # rounded to bf16 once per block, well within the 2e-2 gate.
            # Matmul APs use 127 rows so C row 127 is never read.
            carries = carpool.tile([64, NBLK * D], f32, tag="car")
            hi_t = carpool.tile([64, NBLK * D], bf, tag="hi")

            # Cross-half offset -> carries(0), on ACT so DVE's queue stays a
            # pure build stream at startup.
            ps_off = pspool.tile([64, D], f32, tag="ps")
            for i in range(NPREV):
                nc.tensor.matmul(
                    ps_off[:, :],
                    twp_t[:, ts(i, M2)],
                    xp_tiles[i],
                    start=(i == 0),
                    stop=(i == NPREV - 1),
                )
            nc.scalar.copy(hi_t[:, 0:D], ps_off[:, :])
            nc.scalar.copy(carries[:, 0:D], ps_off[:, :])

            # ---- phase B: blocksums for ALL blocks, upfront, so every
            # carry row is ready early and the carry DMAs can be emitted two
            # blocks ahead of their stores on the serial SP queue (an SP DMA
            # holds the queue for its whole transfer, so a late-emitted
            # carry DMA waits ~11.5us behind a block of stores, gating every
            # matmul of its block).
            # The blocksum matmuls ping-pong two psum tiles and are copied
            # to SBUF fp32 on ACT right away, so the psum-ring slots they
            # occupy free quickly and never gate block 0's matmul tiles.
            # The serial scan adds run on DVE in SBUF (GPSIMD cannot read
            # PSUM on this hardware), deprioritized below the build stream;
            # the bf16 carry-row copies run on Pool.
            bsram = carpool.tile([64, NFULL * D], f32, tag="bsram")
            bs_pp = [
                pspool.tile([64, D], f32, tag="ps", name=f"bs{i}")
                for i in range(2)
            ]
            for j in range(1, NBLK):
                bs = bs_pp[j % 2]
                nc.tensor.matmul(
                    bs[:, :],
                    two_t[0:TB, ts(j - 1, M2)],
                    xo_tiles[j - 1][0:TB],
                    start=True,
                    stop=True,
                )
                nc.scalar.copy(bsram[:, ts(j - 1, D)], bs[:, :])
            prio_save = tc.cur_priority
            tc.cur_priority += 120
            for j in range(1, NBLK):
                nc.vector.tensor_add(
                    carries[:, ts(j, D)],
                    carries[:, ts(j - 1, D)],
                    bsram[:, ts(j - 1, D)],
                )
                nc.gpsimd.tensor_copy(hi_t[:, ts(j, D)], carries[:, ts(j, D)])
            tc.cur_priority = prio_save

            # C half-tiles rotate 2-deep via the pool; tiles for block j+2
            # are allocated during iteration j so their carry-row DMAs sit
            # ahead of stores(j) in SP-queue order.
            def alloc_c(j):
                C_a = cbpool.tile([128, WH], bf, tag="CA", name=f"CA{j}")
                C_b = cbpool.tile([128, WH], bf, tag="CB", name=f"CB{j}")
                return C_a, C_b

            def emit_carry_dma(j, C_a, C_b):
                ch = 126 if j < NFULL else REM
                for C_h, mbase in ((C_a, 0), (C_b, MH)):
                    nc.sync.dma_start(
                        C_h[ch : ch + 1, :].rearrange("p (a b) -> p a b", a=MH),
                        hi_t[mbase : mbase + MH, ts(j, D)],
                    )

            cqueue = []
            for j in range(2):
                Cn = alloc_c(j)
                emit_carry_dma(j, *Cn)
                cqueue.append(Cn)

            # ---- phase C ----
            NPS = WID // PS_FREE   # 16 psum tiles per block
            mm = PS_FREE // D      # 4 m's per psum tile
            pending = []           # deferred DVE converts of the previous block

            def emit_conv(eng, ps_t, o_t, n, rows):
                src_v = ps_t[:rows, :].rearrange("p (mm d) -> p d mm", mm=mm)
                dst = o_t[:rows, :].rearrange("p (d mm) -> p d mm", mm=M2)[
                    :, :, n * mm : (n + 1) * mm
                ]
                if eng == "A":
                    nc.scalar.mul(dst, src_v, NORM)
                else:
                    nc.vector.tensor_scalar_mul(dst, src_v, NORM)

            for j in range(NBLK):
                rows = TB if j < NFULL else REM
                C_a, C_b = cqueue.pop(0)
                for mi in range(M2):
                    C_h = C_a if mi < MH else C_b
                    mh = mi if mi < MH else mi - MH
                    eng = nc.vector if mi < BD else nc.gpsimd
                    eng.tensor_scalar_mul(
                        C_h[0:rows, mh * D : (mh + 1) * D],
                        xo_tiles[j][0:rows],
                        two32_t[0:rows, j * M2 + mi : j * M2 + mi + 1],
                    )
                    # weave the previous block's DVE converts into the build
                    # stream: each drains its psum-ring slot near the ring's
                    # natural pace without stalling the build queue.
                    if mi in WEAVE and WEAVE[mi] < len(pending):
                        emit_conv("D", *pending[WEAVE[mi]])
                o_t = obpool.tile([128, WID], bf, tag="O")
                next_pending = []
                for n in range(NPS):
                    C_h = C_a if n < NPS // 2 else C_b
                    base = 0 if n < NPS // 2 else WH
                    ps_t = pspool.tile([128, PS_FREE], f32, tag="ps", name="ps")
                    for q in range(PS_FREE // 512):
                        col = n * PS_FREE + q * 512 - base
                        if j < NFULL:
                            nc.tensor.matmul(
                                ps_t[:TB, ts(q, 512)],
                                ut_t[0:127, 0:TB],
                                C_h[0:127, col : col + 512],
                                start=True,
                                stop=True,
                            )
                        else:
                            nc.tensor.matmul(
                                ps_t[:REM, ts(q, 512)],
                                ut_t[0 : REM + 1, TB : TB + REM],
                                C_h[0 : REM + 1, col : col + 512],
                                start=True,
                                stop=True,
                            )
                    if CONV_ENG[n] == "A" or DISABLE_WEAVE:
                        emit_conv(CONV_ENG[n], ps_t, o_t, n, rows)
                    else:
                        next_pending.append((ps_t, o_t, n, rows))
                pending = next_pending
                # carry rows for block j+2: ahead of stores(j) in SP order,
                # so they land ~a block before block j+2's matmuls read them.
                if j + 2 < NBLK:
                    Cn = alloc_c(j + 2)
                    emit_carry_dma(j + 2, *Cn)
                    cqueue.append(Cn)
                # stores slice the full-width tile by d-range: both sides
                # contiguous (HBM col = d*64 + m)
                if j == NBLK - 1:
                    for args in pending:
                        emit_conv("D", *args)
                for qq in range(2):
                    nc.sync.dma_start(
                        out_h[
                            j * TB : j * TB + rows,
                            qq * (WID // 2) : (qq + 1) * (WID // 2),
                        ],
                        o_t[:rows, qq * (WID // 2) : (qq + 1) * (WID // 2)],
                    )
    nc.compile()
    return nc


def _host_inputs(x):
    tw = _twiddles_np()
    ut = np.zeros((128, TB + REM), dtype=BF16)
    ut[0:TB, 0:TB] = np.triu(np.ones((TB, TB), dtype=np.float32)).astype(BF16)
    ut[126:128, 0:TB] = 1
    ut[0:REM, TB : TB + REM] = np.triu(np.ones((REM, REM), dtype=np.float32)).astype(
        BF16
    )
    ut[REM : REM + 2, TB : TB + REM] = 1
    twp = np.zeros((128, NPREV * M2), dtype=BF16)
    for i in range(NPREV):
        twp[:, i * M2 : (i + 1) * M2] = tw[i * 128 : (i + 1) * 128, :]
    in_maps = []
    for c in range(NCORES):
        b, h = divmod(c, 2)
        base = h * TH
        xo = np.ascontiguousarray(x[b, base : base + TH, :])
        xp = (
            np.ascontiguousarray(x[b, 0:TH, :])
            if h
            else np.zeros((TH, D), dtype=BF16)
        )
        two = np.zeros((128, NBLK * M2), dtype=BF16)
        for j in range(NBLK):
            rows = TB if j < NFULL else REM
            two[0:rows, j * M2 : (j + 1) * M2] = tw[
                base + j * TB : base + j * TB + rows, :
            ]
        in_maps.append(
            {
                "x_own": xo,
                "x_prev": xp,
                "tw_own": two,
                "tw_own32": two.astype(np.float32),
                "tw_prev": twp,
                "ut": ut,
            }
        )
    return in_maps


def kernel(x):
    global _prog
    x = np.asarray(x)
    assert x.shape == (B, T, D), x.shape
    if x.dtype != BF16:
        x = x.astype(BF16)
    if _prog is None:
        _prog = _build_program()
    from concourse.bass_utils import run_bass_kernel_spmd

    in_maps = _host_inputs(x)
    res = run_bass_kernel_spmd(_prog, in_maps, list(range(NCORES)))
    out = np.empty((B, T, D, K, 2), dtype=BF16)
    for c in range(NCORES):
        b, h = divmod(c, 2)
        out[b, h * TH : (h + 1) * TH] = res.results[c]["out"].reshape(TH, D, K, 2)
    return out
